# revision 20
# baseline (speedup 1.0000x reference)
"""ConvAConnect Trainium2 kernel (v2: bf16, W-stationary / X-moving).

Per-sample noisy conv: Z[b] = conv2d(X[b], W * Werr[b], VALID) + bias * Berr[b].

Data-parallel over batch across 8 NeuronCores (8 samples each). Per core the
conv is 9 shifted matmuls per output tile accumulating in PSUM, with the
operand roles chosen so the moving stream is long and the stationary is a
small weight block:

  psum[cout_half, pix] += memW[kh,kw][cin, cout_half].T @ X[cin, shifted pix]

Operands are bf16 (rel err ~2e-3, gate 2e-2): stationary loads get the
compiler's fast-weight-load (2 elem/cycle, hidden under the moving stream),
and input DMA traffic halves. Output pixels are grouped 8 rows at a time:
the moving X operand is a [cin, rows, 62] strided AP over the [cin, H*W]
X image, so the 2 dead columns of the 64-wide input rows never enter the
PE and each matmul streams N=496 (last group 372) at full rate. PSUM tile
[128, 496] f32 fits one bank; 9 taps accumulate, then the DVE evacuates
with the per-sample bias (bias*Berr, a [128,1] per-partition scalar) fused
in. Output leaves as [sample, cout, pix]; the host transposes back.
"""

import numpy as np

B, H, Wd, CIN, COUT, KH, KW = 64, 64, 64, 128, 256, 3, 3
HO, WO = H - KH + 1, Wd - KW + 1  # 62, 62
NCORES = 8
S = B // NCORES  # samples per core
XF = H * WO  # 3968: X stored per kw-slice as 64 rows x 62 cols
NPIX = HO * WO  # 3844 real output pixels per sample

# output row groups: 7 groups of 8 rows (N=496) + 1 group of 6 rows (N=372)
GROUPS = [(0, 8), (8, 8), (16, 8), (24, 8), (32, 8), (40, 8), (48, 8), (56, 6)]
TAPF = KH * KW * COUT  # 2304: memW free layout [cin, (tap cout)]

TRACE = False  # set by test harness to capture an NTFF profile
LAST_RESULTS = None  # BassKernelResults of the most recent run (for profiling)

_prog_cache = None


def _build_program():
    import concourse.mybir as mybir
    from concourse import bacc
    from concourse.tile import TileContext
    from concourse.tile_rust import add_dep_helper

    f32 = mybir.dt.float32
    bf16 = mybir.dt.bfloat16

    nc = bacc.Bacc()

    # X as 3 kw-sliced copies: X3[kw][cin, r*62+c] = X[r, c+kw, cin] so every
    # tap's moving slab is contiguous with no dead columns
    X_t = nc.declare_dram_parameter("X_t", [S, KW, CIN, XF], bf16, isOutput=False)
    W_p = nc.declare_dram_parameter("W", [CIN, TAPF], bf16, isOutput=False)
    bias_p = nc.declare_dram_parameter("bias", [128, 2], f32, isOutput=False)
    Werr_p = nc.declare_dram_parameter("Werr", [S, CIN, TAPF], bf16, isOutput=False)
    Berr_p = nc.declare_dram_parameter("Berr", [S, 128, 2], f32, isOutput=False)
    # out rows are [cout, pix] per (sample, cout-half); host transposes back
    OUT = nc.declare_dram_parameter("OUT", [S, 2, 128, NPIX], bf16, isOutput=True)

    HEAD = 3 * COUT  # taps 0-2: the startup-critical slice

    with TileContext(nc) as tc:
        with (
            tc.tile_pool(name="const", bufs=1) as cpool,
            tc.tile_pool(name="xp", bufs=2) as xpool,
            tc.tile_pool(name="wep", bufs=2) as wepool,
            tc.tile_pool(name="mwp", bufs=2) as mwpool,
            tc.tile_pool(name="bbp", bufs=2) as bbpool,
            tc.tile_pool(name="outp", bufs=6) as opool,
            tc.tile_pool(name="ps", bufs=7, space="PSUM") as pspool,
            tc.tile_pool(name="psw", bufs=1, space="PSUM") as pswpool,
        ):
            # W taps, resident all run: [cin, (t cout)]; head first so the
            # first memW mul only waits on the first 3 taps
            W_sb = cpool.tile([CIN, TAPF], bf16)
            nc.sync.dma_start(out=W_sb[:, :HEAD], in_=W_p[:, :HEAD])
            nc.sync.dma_start(out=W_sb[:, HEAD:], in_=W_p[:, HEAD:])
            bias_sb = cpool.tile([128, 2], f32)
            nc.gpsimd.dma_start(out=bias_sb, in_=bias_p[:, :])

            # PE pre-warm: dummy bf16 matmuls with no DMA dependency run during
            # the startup DMA window so the HAM clock gate reaches 2.4GHz
            # before the first real matmul (~2.8us of busy at the cold clock).
            warm = cpool.tile([128, 384], bf16)
            nc.vector.memset(warm, 1.0)
            ps_warm = pswpool.tile([128, 256], f32)
            NWARM = 16
            for i in range(NWARM):
                nc.tensor.matmul(
                    ps_warm[:],
                    warm[:, :128],
                    warm[:, 128:],
                    start=(i == 0),
                    stop=(i == NWARM - 1),
                )

            s0_last_werr = None
            for s in range(S):
                # X rows 0..15 of each kw-slice first: group 0 needs rows 0..9
                X_sb = xpool.tile([CIN, KW * XF], bf16)
                XH = 16 * WO
                for kw in range(KW):
                    xp_dma = nc.sync.dma_start(
                        out=X_sb[:, kw * XF : kw * XF + XH], in_=X_t[s, kw, :, :XH]
                    )
                    if s == 1 and kw == 0 and s0_last_werr is not None:
                        # hold the s1 prefetch until s0's Werr has fully
                        # landed: the DMA fabric round-robins packets across
                        # outstanding transfers, so an early prefetch starves
                        # s0's startup-critical loads
                        add_dep_helper(
                            xp_dma.ins,
                            s0_last_werr.ins,
                            sync=True,
                            reason="s1 prefetch yields bandwidth to s0 startup",
                        )

                # Werr in 3 tap-triples; memW muls chase the pieces
                Werr_sb = wepool.tile([CIN, TAPF], bf16)
                memW = mwpool.tile([CIN, TAPF], bf16)
                for g3 in range(3):
                    lo, hi = g3 * HEAD, (g3 + 1) * HEAD
                    wdma = nc.sync.dma_start(out=Werr_sb[:, lo:hi], in_=Werr_p[s, :, lo:hi])
                    nc.vector.tensor_mul(memW[:, lo:hi], W_sb[:, lo:hi], Werr_sb[:, lo:hi])
                    if s == 0:
                        s0_last_werr = wdma

                for kw in range(KW):
                    xt_dma = nc.sync.dma_start(
                        out=X_sb[:, kw * XF + XH : (kw + 1) * XF],
                        in_=X_t[s, kw, :, XH:],
                    )
                    if s == 0 and s0_last_werr is not None:
                        # X tails aren't needed until group 2; keep them off
                        # the fabric while s0's startup-critical Werr streams
                        add_dep_helper(
                            xt_dma.ins,
                            s0_last_werr.ins,
                            sync=True,
                            reason="s0 X tail yields bandwidth to s0 Werr",
                        )

                berr_sb = bbpool.tile([128, 2], f32)
                nc.gpsimd.dma_start(out=berr_sb, in_=Berr_p[s, :, :])
                membias = bbpool.tile([128, 2], f32)
                nc.vector.tensor_mul(membias, bias_sb, berr_sb)

                for r0, nr in GROUPS:
                    npix = nr * WO
                    for h in range(2):
                        ps = pspool.tile([128, npix], f32, tag="ps")
                        for t in range(KH * KW):
                            kh, kw = divmod(t, KW)
                            # moving X: contiguous slab of the kw-slice
                            base = kw * XF + (r0 + kh) * WO
                            rhs = X_sb[:, base : base + npix]
                            lhsT = memW[:, t * COUT + h * 128 : t * COUT + h * 128 + 128]
                            nc.tensor.matmul(
                                ps[:],
                                lhsT,
                                rhs,
                                start=(t == 0),
                                stop=(t == KH * KW - 1),
                            )
                        o_sb = opool.tile([128, npix], bf16)
                        nc.vector.tensor_scalar_add(o_sb, ps, membias[:, h : h + 1])
                        nc.scalar.dma_start(
                            out=OUT[s, h, :, r0 * WO : r0 * WO + npix], in_=o_sb
                        )

    nc.compile()
    return nc


def _get_program():
    global _prog_cache
    if _prog_cache is None:
        _prog_cache = _build_program()
    return _prog_cache


def kernel(X, W, bias, Werr, Berr):
    global LAST_RESULTS
    import ml_dtypes
    from concourse.bass_utils import run_bass_kernel_spmd

    bf16 = ml_dtypes.bfloat16
    X = np.asarray(X, dtype=np.float32)
    W = np.asarray(W, dtype=np.float32)
    bias = np.asarray(bias, dtype=np.float32)
    Werr = np.asarray(Werr, dtype=np.float32)
    Berr = np.asarray(Berr, dtype=np.float32)

    # host-side layout prep (part of sharding): Cin onto partitions; 3
    # kw-sliced 62-wide copies so every tap slab is contiguous on device
    Xc = X.transpose(0, 3, 1, 2).astype(bf16)  # [B, cin, H, Wd]
    X_t = np.empty((B, KW, CIN, XF), dtype=bf16)
    for kw in range(KW):
        X_t[:, kw] = Xc[:, :, :, kw : kw + WO].reshape(B, CIN, XF)
    # [kh,kw,cin,cout] -> [cin, (tap cout)]
    W2 = np.ascontiguousarray(
        W.reshape(KH * KW, CIN, COUT).transpose(1, 0, 2).reshape(CIN, TAPF)
    ).astype(bf16)
    Werr2 = np.ascontiguousarray(
        Werr.reshape(B, KH * KW, CIN, COUT).transpose(0, 2, 1, 3).reshape(B, CIN, TAPF)
    ).astype(bf16)
    # bias/Berr as [128 partitions, 2 halves]
    bias2 = np.ascontiguousarray(bias.reshape(2, 128).T)
    Berr2 = np.ascontiguousarray(Berr.reshape(B, 2, 128).transpose(0, 2, 1))

    nc = _get_program()
    in_maps = []
    for core in range(NCORES):
        sl = slice(core * S, (core + 1) * S)
        in_maps.append(
            {
                "X_t": X_t[sl],
                "W": W2,
                "bias": bias2,
                "Werr": Werr2[sl],
                "Berr": Berr2[sl],
            }
        )

    res = run_bass_kernel_spmd(nc, in_maps, core_ids=list(range(NCORES)), trace=TRACE)
    LAST_RESULTS = res
    out = np.concatenate([r["OUT"] for r in res.results], axis=0)  # [B,2,128,NPIX]
    # [B, cout, pix] -> [B, ho, wo, cout]
    return np.ascontiguousarray(
        out.reshape(B, COUT, HO, WO).transpose(0, 2, 3, 1).astype(np.float32)
    )


# revision 26
# speedup vs baseline: 1.0164x; 1.0164x over previous
"""ConvAConnect Trainium2 kernel (v2: bf16, W-stationary / X-moving).

Per-sample noisy conv: Z[b] = conv2d(X[b], W * Werr[b], VALID) + bias * Berr[b].

Data-parallel over batch across 8 NeuronCores (8 samples each). Per core the
conv is 9 shifted matmuls per output tile accumulating in PSUM, with the
operand roles chosen so the moving stream is long and the stationary is a
small weight block:

  psum[cout_half, pix] += memW[kh,kw][cin, cout_half].T @ X[cin, shifted pix]

Operands are bf16 (rel err ~2e-3, gate 2e-2): stationary loads get the
compiler's fast-weight-load (2 elem/cycle, hidden under the moving stream),
and input DMA traffic halves. Output pixels are grouped 8 rows at a time:
the moving X operand is a [cin, rows, 62] strided AP over the [cin, H*W]
X image, so the 2 dead columns of the 64-wide input rows never enter the
PE and each matmul streams N=496 (last group 372) at full rate. PSUM tile
[128, 496] f32 fits one bank; 9 taps accumulate, then the DVE evacuates
with the per-sample bias (bias*Berr, a [128,1] per-partition scalar) fused
in. Output leaves as [sample, cout, pix]; the host transposes back.
"""

import numpy as np

B, H, Wd, CIN, COUT, KH, KW = 64, 64, 64, 128, 256, 3, 3
HO, WO = H - KH + 1, Wd - KW + 1  # 62, 62
NCORES = 8
S = B // NCORES  # samples per core
XF = H * WO  # 3968: X stored per kw-slice as 64 rows x 62 cols
NPIX = HO * WO  # 3844 real output pixels per sample

# output row groups: 7 groups of 8 rows (N=496) + 1 group of 6 rows (N=372)
GROUPS = [(0, 8), (8, 8), (16, 8), (24, 8), (32, 8), (40, 8), (48, 8), (56, 6)]
TAPF = KH * KW * COUT  # 2304: memW free layout [cin, (tap cout)]

TRACE = False  # set by test harness to capture an NTFF profile
LAST_RESULTS = None  # BassKernelResults of the most recent run (for profiling)

_prog_cache = None


def _build_program():
    import concourse.mybir as mybir
    from concourse import bacc
    from concourse.tile import TileContext
    from concourse.tile_rust import add_dep_helper

    f32 = mybir.dt.float32
    bf16 = mybir.dt.bfloat16
    u8 = mybir.dt.uint8

    nc = bacc.Bacc()

    # X as 3 kw-sliced copies: X3[kw][cin, r*62+c] = X[r, c+kw, cin] so every
    # tap's moving slab is contiguous with no dead columns
    X_t = nc.declare_dram_parameter("X_t", [S, KW, CIN, XF], bf16, isOutput=False)
    W_p = nc.declare_dram_parameter("W", [CIN, TAPF], bf16, isOutput=False)
    bias_p = nc.declare_dram_parameter("bias", [128, 2], f32, isOutput=False)
    # Werr rides as uint8 (Werr >= 0; host folds the dequant scale into W)
    # halving the startup-critical and per-sample weight-noise traffic
    Werr_p = nc.declare_dram_parameter("Werr", [S, CIN, TAPF], u8, isOutput=False)
    Berr_p = nc.declare_dram_parameter("Berr", [S, 128, 2], f32, isOutput=False)
    # out rows are [cout, pix] per (sample, cout-half); host transposes back
    OUT = nc.declare_dram_parameter("OUT", [S, 2, 128, NPIX], bf16, isOutput=True)

    HEAD = 3 * COUT  # taps 0-2: the startup-critical slice

    with TileContext(nc) as tc:
        with (
            tc.tile_pool(name="const", bufs=1) as cpool,
            tc.tile_pool(name="xp", bufs=2) as xpool,
            tc.tile_pool(name="wep", bufs=2) as wepool,
            tc.tile_pool(name="mwp", bufs=2) as mwpool,
            tc.tile_pool(name="bbp", bufs=2) as bbpool,
            tc.tile_pool(name="outp", bufs=6) as opool,
            tc.tile_pool(name="ps", bufs=7, space="PSUM") as pspool,
            tc.tile_pool(name="psw", bufs=1, space="PSUM") as pswpool,
        ):
            # W taps, resident all run: [cin, (t cout)]; head first so the
            # first memW mul only waits on the first 3 taps
            W_sb = cpool.tile([CIN, TAPF], bf16)
            nc.sync.dma_start(out=W_sb[:, :HEAD], in_=W_p[:, :HEAD])
            nc.sync.dma_start(out=W_sb[:, HEAD:], in_=W_p[:, HEAD:])
            bias_sb = cpool.tile([128, 2], f32)
            nc.gpsimd.dma_start(out=bias_sb, in_=bias_p[:, :])

            # PE pre-warm: dummy bf16 matmuls with no DMA dependency run during
            # the startup DMA window so the HAM clock gate reaches 2.4GHz
            # before the first real matmul (~2.8us of busy at the cold clock).
            warm = cpool.tile([128, 384], bf16)
            nc.vector.memset(warm, 1.0)
            ps_warm = pswpool.tile([128, 256], f32)
            NWARM = 40
            for i in range(NWARM):
                nc.tensor.matmul(
                    ps_warm[:],
                    warm[:, :128],
                    warm[:, 128:],
                    start=(i == 0),
                    stop=(i == NWARM - 1),
                )

            s0_last_werr = None
            for s in range(S):
                # X rows 0..15 of each kw-slice first: group 0 needs rows 0..9
                X_sb = xpool.tile([CIN, KW * XF], bf16)
                XH = 16 * WO
                for kw in range(KW):
                    xp_dma = nc.sync.dma_start(
                        out=X_sb[:, kw * XF : kw * XF + XH], in_=X_t[s, kw, :, :XH]
                    )
                    if s == 1 and kw == 0 and s0_last_werr is not None:
                        # hold the s1 prefetch until s0's Werr has fully
                        # landed: the DMA fabric round-robins packets across
                        # outstanding transfers, so an early prefetch starves
                        # s0's startup-critical loads
                        add_dep_helper(
                            xp_dma.ins,
                            s0_last_werr.ins,
                            sync=True,
                            reason="s1 prefetch yields bandwidth to s0 startup",
                        )

                # Werr in 3 tap-triples; memW muls chase the pieces
                Werr_sb = wepool.tile([CIN, TAPF], u8)
                memW = mwpool.tile([CIN, TAPF], bf16)
                for g3 in range(3):
                    lo, hi = g3 * HEAD, (g3 + 1) * HEAD
                    wdma = nc.sync.dma_start(out=Werr_sb[:, lo:hi], in_=Werr_p[s, :, lo:hi])
                    nc.vector.tensor_mul(memW[:, lo:hi], W_sb[:, lo:hi], Werr_sb[:, lo:hi])
                    if s == 0:
                        s0_last_werr = wdma

                for kw in range(KW):
                    nc.sync.dma_start(
                        out=X_sb[:, kw * XF + XH : (kw + 1) * XF],
                        in_=X_t[s, kw, :, XH:],
                    )

                berr_sb = bbpool.tile([128, 2], f32)
                nc.gpsimd.dma_start(out=berr_sb, in_=Berr_p[s, :, :])
                membias = bbpool.tile([128, 2], f32)
                nc.vector.tensor_mul(membias, bias_sb, berr_sb)

                for r0, nr in GROUPS:
                    npix = nr * WO
                    for h in range(2):
                        ps = pspool.tile([128, npix], f32, tag="ps")
                        for t in range(KH * KW):
                            kh, kw = divmod(t, KW)
                            # moving X: contiguous slab of the kw-slice
                            base = kw * XF + (r0 + kh) * WO
                            rhs = X_sb[:, base : base + npix]
                            lhsT = memW[:, t * COUT + h * 128 : t * COUT + h * 128 + 128]
                            nc.tensor.matmul(
                                ps[:],
                                lhsT,
                                rhs,
                                start=(t == 0),
                                stop=(t == KH * KW - 1),
                            )
                        o_sb = opool.tile([128, npix], bf16)
                        nc.vector.tensor_scalar_add(o_sb, ps, membias[:, h : h + 1])
                        nc.scalar.dma_start(
                            out=OUT[s, h, :, r0 * WO : r0 * WO + npix], in_=o_sb
                        )

    nc.compile()
    return nc


def _get_program():
    global _prog_cache
    if _prog_cache is None:
        _prog_cache = _build_program()
    return _prog_cache


def kernel(X, W, bias, Werr, Berr):
    global LAST_RESULTS
    import ml_dtypes
    from concourse.bass_utils import run_bass_kernel_spmd

    bf16 = ml_dtypes.bfloat16
    X = np.asarray(X, dtype=np.float32)
    W = np.asarray(W, dtype=np.float32)
    bias = np.asarray(bias, dtype=np.float32)
    Werr = np.asarray(Werr, dtype=np.float32)
    Berr = np.asarray(Berr, dtype=np.float32)

    # host-side layout prep (part of sharding): Cin onto partitions; 3
    # kw-sliced 62-wide copies so every tap slab is contiguous on device
    Xc = X.transpose(0, 3, 1, 2).astype(bf16)  # [B, cin, H, Wd]
    X_t = np.empty((B, KW, CIN, XF), dtype=bf16)
    for kw in range(KW):
        X_t[:, kw] = Xc[:, :, :, kw : kw + WO].reshape(B, CIN, XF)
    # [kh,kw,cin,cout] -> [cin, (tap cout)]; Werr quantized to uint8 with the
    # dequant scale folded into W (memW = (W*s) * round(Werr/s))
    ws = float(Werr.max()) / 255.0
    W2 = np.ascontiguousarray(
        (W * ws).reshape(KH * KW, CIN, COUT).transpose(1, 0, 2).reshape(CIN, TAPF)
    ).astype(bf16)
    Werr2 = np.ascontiguousarray(
        np.clip(np.rint(Werr / ws), 0, 255)
        .astype(np.uint8)
        .reshape(B, KH * KW, CIN, COUT)
        .transpose(0, 2, 1, 3)
        .reshape(B, CIN, TAPF)
    )
    # bias/Berr as [128 partitions, 2 halves]
    bias2 = np.ascontiguousarray(bias.reshape(2, 128).T)
    Berr2 = np.ascontiguousarray(Berr.reshape(B, 2, 128).transpose(0, 2, 1))

    nc = _get_program()
    in_maps = []
    for core in range(NCORES):
        sl = slice(core * S, (core + 1) * S)
        in_maps.append(
            {
                "X_t": X_t[sl],
                "W": W2,
                "bias": bias2,
                "Werr": Werr2[sl],
                "Berr": Berr2[sl],
            }
        )

    res = run_bass_kernel_spmd(nc, in_maps, core_ids=list(range(NCORES)), trace=TRACE)
    LAST_RESULTS = res
    out = np.concatenate([r["OUT"] for r in res.results], axis=0)  # [B,2,128,NPIX]
    # [B, cout, pix] -> [B, ho, wo, cout]
    return np.ascontiguousarray(
        out.reshape(B, COUT, HO, WO).transpose(0, 2, 3, 1).astype(np.float32)
    )
